# revision 5
# baseline (speedup 1.0000x reference)
"""BiGCN layer kernel for 8 Trainium2 NeuronCores.

Strategy (1D column-parallel SpMM, fp16 streams, pipelined ReduceScatter
epilogue):
  - Each core c owns the contraction slice n in [c*512, (c+1)*512) of all six
    adjacency matrices (3 bw + 3 fw), pre-transposed on host to [n_loc, m]
    fp16 so the contraction dim lands on SBUF partitions with no on-chip
    transposes. fp16 halves the dominant HBM traffic; its 11-bit mantissa
    matches the fp32r (TF32-like) precision class for these [0,1) values.
  - The m columns are host-permuted so every 1024-wide compute chunk contains
    a 128-col slab of EVERY destination core's m-block. Partial feats^T
    therefore reduces in m-slabs: ncfw collectives have a ~12-25us per-op
    floor (first op also absorbs cross-core launch skew), so exactly three
    are used — one 2MB RS for the whole bw direction (hides under the fw
    stream and soaks the skew) and two 1MB RS for fw halves, the first
    hiding under the second half's matmuls. Only the last ~15us RS is
    exposed.
  - sup[r] = inps @ W[r] is computed locally per core for its n-slice.
    inpsT+wst load FIRST on the sync DMA ring with the adjacency stream
    force-ordered behind them (two HWDGE rings share the 16 SDMA engines, so
    a separate queue would be starved to ~25% bandwidth by the stream).
  - bias+relu fuse into one scalar-engine activation per 128-row tile; the
    final linear runs in fp32r per 128-col slab as each RS lands, and the
    residual adds an exact fp32 copy of inps^T. Host assembles the 8
    transposed output blocks.
"""

import numpy as np

N, H, R = 4096, 512, 3
K = H // 2            # 256
NC = 8                # cores
NB = N // NC          # 512 rows (m / n_loc) per core
MC = 1024             # m-chunk width per PSUM accumulation group
NMC = N // MC         # 4 compute chunks per direction
UB = MC // NC         # 128 dest-block cols per compute chunk

_BUILT = {}


def _build_nc():
    """Build (and cache) the Bass program. Identical program on all 8 cores."""
    if "nc" in _BUILT:
        return _BUILT["nc"]

    import concourse.bass as bass
    import concourse.mybir as mybir
    from concourse import bacc, tile
    from concourse.tile import add_dep_helper

    f32 = mybir.dt.float32
    f32r = mybir.dt.float32r
    f16 = mybir.dt.float16
    nc = bacc.Bacc(None, num_devices=NC)

    inpsT = nc.dram_tensor("inpsT", [H, NB], f16, kind="ExternalInput")
    inpsR = nc.dram_tensor("inpsR", [H, NB], f32, kind="ExternalInput")
    # [rel, chunk, p, nt, m'] — host-permuted m columns, 8KB per partition
    # per chunk so each 1MB chunk DMA is one descriptor per partition.
    adjd = nc.dram_tensor(
        "adjd", [2 * R, NMC, 128, NB // 128, MC], f16, kind="ExternalInput"
    )
    wst = nc.dram_tensor("wst", [2 * R, H, K], f16, kind="ExternalInput")
    bstack = nc.dram_tensor("bstack", [4, 128, R], f32, kind="ExternalInput")
    w1 = nc.dram_tensor("w1", [H, H], f32r, kind="ExternalInput")
    b1s = nc.dram_tensor("b1s", [4, 128, 1], f32, kind="ExternalInput")
    outT = nc.dram_tensor("outT", [H, NB], f32, kind="ExternalOutput")

    HT = H // 128     # 4 h-tiles
    NT = NB // 128    # 4 n_loc tiles
    JT = H // 128     # 4 output j tiles
    Relu = mybir.ActivationFunctionType.Relu
    Identity = mybir.ActivationFunctionType.Identity

    with tile.TileContext(nc) as tc:
        with (
            tc.tile_pool(name="const", bufs=1) as const,
            tc.tile_pool(name="adjp", bufs=9) as adjp,
            tc.tile_pool(name="evacp", bufs=4) as evacp,
            tc.tile_pool(name="ftp", bufs=4) as ftp,
            tc.tile_pool(name="otp", bufs=2) as otp,
            tc.tile_pool(name="psum", bufs=3, space=bass.MemorySpace.PSUM) as psump,
            tc.tile_pool(name="psumF", bufs=2, space=bass.MemorySpace.PSUM) as psumF,
            tc.tile_pool(name="dram", bufs=1, space="DRAM") as dramp,
        ):
            # ---------------- constants into SBUF ----------------
            # sup-critical tensors go FIRST on the sync ring; every adjacency
            # chunk DMA is force-ordered behind them so the scheduler cannot
            # starve the supports.
            inpsT_sb = const.tile([128, HT, NB], f16)       # [p_h, ht, n_loc]
            nc.sync.dma_start(inpsT_sb[:], inpsT[:, :].rearrange("(t p) n -> p t n", p=128))
            wst_sb = const.tile([128, 2, R, HT, K], f16)    # [p_h, dir, r, ht, k]
            last_const = None
            for dirn in range(2):
                last_const = nc.sync.dma_start(
                    wst_sb[:, dirn],
                    wst[dirn * R : (dirn + 1) * R].rearrange("r (t p) k -> p r t k", p=128),
                )
            # non-urgent constants trickle on the (stream-starved) scalar ring
            w1_sb = const.tile([128, HT, H], f32r)          # [p_h, ht, j]
            nc.scalar.dma_start(w1_sb[:], w1[:, :].rearrange("(t p) j -> p t j", p=128))
            bst_sb = const.tile([128, JT, R], f32)
            nc.scalar.dma_start(bst_sb[:], bstack[:, :, :].rearrange("t p r -> p t r"))
            b1_sb = const.tile([128, JT], f32)
            nc.scalar.dma_start(b1_sb[:], b1s[:, :, :].rearrange("t p o -> p (t o)"))
            inpsR_sb = const.tile([128, HT, NB], f32)       # exact fp32 for residual
            nc.scalar.dma_start(inpsR_sb[:], inpsR[:, :].rearrange("(t p) n -> p t n", p=128))

            # summed (over relations) concat bias, per (p, ht)
            bias_sb = const.tile([128, JT], f32)
            for jt in range(JT):
                nc.vector.tensor_add(
                    bias_sb[:, jt : jt + 1], bst_sb[:, jt, 0:1], bst_sb[:, jt, 1:2]
                )
                nc.vector.tensor_add(
                    bias_sb[:, jt : jt + 1], bias_sb[:, jt : jt + 1], bst_sb[:, jt, 2:3]
                )

            # ---------------- local supports: sup[r][n_loc, k] ----------------
            sup_sb = const.tile([128, 2, R, NT, K], f16)    # [p_n, dir, r, nt, k]

            def emit_sup(dirn):
                for ri0, nr in ((0, 2), (2, 1)):            # pair + single
                    for nt in range(NT):
                        ps = psump.tile([128, nr * K], f32, tag="pb", name="psup")
                        for ht in range(HT):
                            nc.tensor.matmul(
                                ps[:],
                                inpsT_sb[:, ht, nt * 128 : (nt + 1) * 128],
                                wst_sb[:, dirn, ri0 : ri0 + nr, ht, :],
                                start=(ht == 0),
                                stop=(ht == HT - 1),
                            )
                        nc.vector.tensor_copy(sup_sb[:, dirn, ri0 : ri0 + nr, nt, :], ps[:])

            # ---------------- staging + RS tensors ----------------
            # bw: one RS over the whole direction (absorbs ncfw init + core
            # launch skew under the fw stream). fw: two half RS; only the
            # second is tail-exposed.
            stag_bw = dramp.tile([NC, 2 * 128, NB], f16, name="stb", tag="stb")
            rso_bw = dramp.tile([1, 2 * 128, NB], f16, name="rob", tag="rob")
            stag_fw = [
                dramp.tile([NC, 2 * 128, 2 * UB], f16, name=f"stf{i}", tag=f"stf{i}")
                for i in range(2)
            ]
            rso_fw = [
                dramp.tile([1, 2 * 128, 2 * UB], f16, name=f"rof{i}", tag=f"rof{i}")
                for i in range(2)
            ]

            def emit_rs(src, dst):
                nc.gpsimd.collective_compute(
                    "ReduceScatter",
                    mybir.AluOpType.add,
                    replica_groups=[list(range(NC))],
                    ins=[src[:].opt()],
                    outs=[dst[:].opt()],
                )

            # ---------------- adjacency stream ----------------
            for dirn in range(2):                           # 0 = bw (h 0:256), 1 = fw
                emit_sup(dirn)
                for q in range(NMC):
                    ps0 = psump.tile([128, MC], f32, tag="pb", name="ps0")  # k 0:128
                    ps1 = psump.tile([128, MC], f32, tag="pb", name="ps1")  # k 128:256
                    for ri in range(R):
                        r = dirn * R + ri
                        at = adjp.tile([128, NT, MC], f16, tag="adj")
                        d = nc.sync.dma_start(at[:], adjd[r, q])
                        add_dep_helper(
                            d.ins, last_const.ins, reason="adj stream after consts"
                        )
                        for nt in range(NT):
                            first = ri == 0 and nt == 0
                            last = ri == R - 1 and nt == NT - 1
                            for kk, ps in ((0, ps0), (1, ps1)):
                                lhsT = sup_sb[:, dirn, ri, nt, kk * 128 : (kk + 1) * 128]
                                for mh in range(MC // 512):
                                    nc.tensor.matmul(
                                        ps[:, mh * 512 : (mh + 1) * 512],
                                        lhsT,
                                        at[:, nt, mh * 512 : (mh + 1) * 512],
                                        start=first,
                                        stop=last,
                                    )
                    for kk, ps in ((0, ps0), (1, ps1)):
                        ev = evacp.tile([128, MC], f16, tag="ev")
                        nc.vector.tensor_copy(ev[:], ps[:])
                        if dirn == 0:
                            dest = stag_bw[:, kk * 128 : (kk + 1) * 128, q * UB : (q + 1) * UB]
                        else:
                            dest = stag_fw[q // 2][
                                :, kk * 128 : (kk + 1) * 128, (q % 2) * UB : (q % 2 + 1) * UB
                            ]
                        nc.scalar.dma_start(
                            dest.rearrange("d p c -> p d c"),
                            ev[:].rearrange("p (d c) -> p d c", d=NC),
                        )
                    if dirn == 0 and q == NMC - 1:
                        emit_rs(stag_bw, rso_bw)
                    elif dirn == 1 and q % 2 == 1:
                        emit_rs(stag_fw[q // 2], rso_fw[q // 2])

            # ---------------- bias + relu + final linear + residual ----------------
            # frelu [p, ht, block-col]; bw rows (ht 0,1) land early from the
            # direction RS; each fw half completes two 128-col slabs, which
            # run the small final matmul, bias, residual, and stream out.
            frelu_sb = const.tile([128, HT, NB], f32r)
            ftb = ftp.tile([128, 2, NB], f16, tag="ftb")
            nc.scalar.dma_start(ftb[:], rso_bw[0].rearrange("(t p) c -> p t c", p=128))
            for t in range(2):
                nc.scalar.activation(
                    frelu_sb[:, t, :], ftb[:, t, :], Relu, bias=bias_sb[:, t : t + 1]
                )
            for i in range(2):                              # fw halves
                ftf = ftp.tile([128, 2, 2 * UB], f16, tag="ftf")
                nc.scalar.dma_start(
                    ftf[:], rso_fw[i][0].rearrange("(t p) c -> p t c", p=128)
                )
                for t in range(2):
                    nc.scalar.activation(
                        frelu_sb[:, 2 + t, i * 2 * UB : (i + 1) * 2 * UB],
                        ftf[:, t, :],
                        Relu,
                        bias=bias_sb[:, 2 + t : 3 + t],
                    )
                for q in (2 * i, 2 * i + 1):                # 128-col slabs
                    # jt outer: start=True clears the whole PSUM bank's
                    # has_written bits, so each jt's accumulation must finish
                    # before the next jt's start (completed data survives).
                    pso = psumF.tile([128, JT, UB], f32, tag="pf", name="pso")
                    for jt in range(JT):
                        for ht in range(HT):
                            nc.tensor.matmul(
                                pso[:, jt, :],
                                w1_sb[:, ht, jt * 128 : (jt + 1) * 128],
                                frelu_sb[:, ht, q * UB : (q + 1) * UB],
                                start=(ht == 0),
                                stop=(ht == HT - 1),
                            )
                    ot = otp.tile([128, JT, UB], f32, tag="ot")
                    for jt in range(JT):
                        nc.scalar.activation(
                            ot[:, jt, :], pso[:, jt, :], Identity, bias=b1_sb[:, jt : jt + 1]
                        )
                    nc.vector.tensor_add(
                        ot[:], ot[:], inpsR_sb[:, :, q * UB : (q + 1) * UB]
                    )
                    nc.sync.dma_start(
                        outT[:, q * UB : (q + 1) * UB].rearrange("(t p) c -> p t c", p=128),
                        ot[:],
                    )

    nc.compile()
    nc.finalize()
    _BUILT["nc"] = nc
    return nc


def _round_fp32r(a):
    """Round fp32 to the fp32r (TF32-like, 1s+8e+11m in top 20 bits) format
    with round-to-nearest-even, as the PE's fp32r datapath expects."""
    b = np.ascontiguousarray(a, np.float32).view(np.uint32).astype(np.uint64)
    lsb = (b >> 12) & 1
    r = ((b + 0x7FF + lsb) & 0xFFFFF000).astype(np.uint32)
    return r.view(np.float32)


def _perm_m():
    """Column permutation: m' = q*1024 + d*128 + c  <-  m = d*512 + q*128 + c,
    so each 1024-wide compute chunk holds a 128-col slab of every dest block."""
    q = np.arange(NMC)[:, None, None]
    d = np.arange(NC)[None, :, None]
    c = np.arange(UB)[None, None, :]
    return (d * NB + q * UB + c).reshape(-1)


def _make_in_maps(inps, fw_adjs, bw_adjs, W_fw, b_fw, W_bw, b_bw, W1, b1):
    f = np.float32
    inps = np.asarray(inps, f)
    W1 = _round_fp32r(np.asarray(W1, f))
    wst = np.ascontiguousarray(
        np.concatenate([np.asarray(W_bw, f), np.asarray(W_fw, f)], axis=0),
        np.float16,
    )
    b_cat = np.concatenate([np.asarray(b_bw, f), np.asarray(b_fw, f)], axis=1)  # [R, H]
    bstack = np.ascontiguousarray(b_cat.T.reshape(4, 128, R))
    b1s = np.ascontiguousarray(np.asarray(b1, f).reshape(4, 128, 1))
    fw_adjs = np.asarray(fw_adjs, f)
    bw_adjs = np.asarray(bw_adjs, f)
    perm = _perm_m()

    in_maps = []
    for c in range(NC):
        sl = slice(c * NB, (c + 1) * NB)
        adjd_c = np.empty((2 * R, NMC, 128, NB // 128, MC), np.float16)
        for r in range(R):
            for idx, adjs in ((r, bw_adjs), (R + r, fw_adjs)):
                # [n_loc, m'] -> [q, p, nt, mc]
                at = adjs[r][perm, sl].T.astype(np.float16)      # [NB, N] permuted
                adjd_c[idx] = at.reshape(NB // 128, 128, NMC, MC).transpose(2, 1, 0, 3)
        in_maps.append(
            {
                "inpsT": np.ascontiguousarray(inps[sl].T, np.float16),
                "inpsR": np.ascontiguousarray(inps[sl].T),
                "adjd": adjd_c,
                "wst": wst,
                "bstack": bstack,
                "w1": W1,
                "b1s": b1s,
            }
        )
    return in_maps


def run(trace=False, **inputs):
    """Run the SPMD kernel; returns (full_output, BassKernelResults)."""
    from concourse.bass_utils import run_bass_kernel_spmd

    nc = _build_nc()
    in_maps = _make_in_maps(**inputs)
    res = run_bass_kernel_spmd(nc, in_maps, core_ids=list(range(NC)), trace=trace)
    out = np.empty((N, H), np.float32)
    for c in range(NC):
        out[c * NB : (c + 1) * NB] = res.results[c]["outT"].T
    return out, res


def kernel(**inputs):
    # Collective-heavy SPMD runs have shown a rare corrupted execution
    # (launch-skew related). Executions are cheap next to compile, so run
    # twice and accept only agreeing results, with a third as tiebreaker.
    out1, _ = run(trace=False, **inputs)
    out2, _ = run(trace=False, **inputs)
    if np.array_equal(out1, out2):
        return out1
    out3, _ = run(trace=False, **inputs)
    return out3 if np.array_equal(out2, out3) else out1
